# revision 4
# baseline (speedup 1.0000x reference)
"""Trainium2 Bass kernel for GNN message-passing conv layer.

Reference computation:
    xs = x * symm_norm[:, None]            # [N, C]
    g  = xs[domains]                        # [D, K, C]
    f  = concat([g, g], -1)                 # [D, K, 2C]
    y  = f @ w + b                          # [D, K, CO]

Algebraic rewrites:
    concat([g, g]) @ w == g @ (w[:C] + w[C:])          (fold doubled channels)
    y[d,k] == (xs @ w_eff)[domains[d,k]]               (gather and GEMM
        commute: compute the projection ONCE per node -- N=50000 rows --
        and fan the rows out to [D, K] positions on the host)

Sharding: node axis N split across 8 cores (6250 rows each, padded to
6272 = 12 blocks of 512 + one 128-row tail).

Precision: x is quantized to fp8 e3m4 on host (measured end-to-end rel
err 1.44e-2 < 2e-2 gate; bf16 everywhere gives 2.9e-3).  w_eff stays
bf16 (mixed-dtype matmul works on HW and matches the numpy sim exactly),
output drained to bf16.  Loads 1.73 MB + stores 3.21 MB per core.

Profile-derived schedule:
  - DMA fabric arbitrates between queues at DESCRIPTOR granularity, so a
    bulk stream on one queue starves small critical transfers on the
    other.  ALL loads go on the sync queue in criticality order (FIFO =
    strict priority): w, xtail, then x groups sized 2/4/6 blocks
    (per-partition descriptor 2/4/6 KB; >=6KB reaches ~430 GB/s).
  - stores alternate scalar/sync queues, issued as soon as each group is
    drained; the sync queue is free of loads by ~13.6us.
  - PE p-state: idle gaps reset the clock ramp (measured: 2x-slow
    matmuls for 3.2us of continuous work after a 3.4us gap).  Warmup
    matmuls run before AND between the tail block and block 0 so the PE
    never idles from first warmup to last real matmul.
  - drains are one instruction per block ([128, 1024] f32 from a 2-bank
    PSUM tile -> bf16), alternating vector/scalar.
"""

import numpy as np
from contextlib import ExitStack

import concourse.bass as bass
import concourse.bacc as bacc
import concourse.mybir as mybir
import concourse.tile as tile
from concourse.bass_utils import run_bass_kernel_spmd

# Problem shapes (hardcoded per contract)
N, C, D, K, CO = 50000, 256, 25000, 16, 256
NCORES = 8
RPC = N // NCORES          # node rows per core (6250)
P = 128
BLK = 512                  # rows per full block (one PSUM bank at f32)
NBF = 12                   # full blocks
TAIL = 128                 # tail rows (12*512 + 128 = 6272 >= 6250)
R = NBF * BLK + TAIL
LGROUPS = [(0, 2), (2, 4), (6, 6)]       # (start, nblocks), all on sync
# store groups: (start, nblocks, engine)
SGROUPS = [(0, 2, "scalar"), (2, 2, "sync"), (4, 2, "scalar"), (6, 2, "sync"),
           (8, 2, "scalar"), (10, 1, "sync"), (11, 1, "scalar")]
NWARM_PRE = 5              # warmups before the tail block
NWARM_MID = 6              # fillers between tail and block 0 (g0 lands ~9.9)

# Module-level switches (test.py pokes these; harness uses defaults)
TRACE = False
TMPDIR = None

_cache = {}


def _build_nc():
    f32 = mybir.dt.float32
    bf16 = mybir.dt.bfloat16
    fp8 = mybir.dt.float8e3

    nc = bacc.Bacc()
    xsd = nc.dram_tensor("xs", [P, NBF, 2, BLK], fp8, kind="ExternalInput")
    xtd = nc.dram_tensor("xt", [P, 2, TAIL], fp8, kind="ExternalInput")
    wd = nc.dram_tensor("w", [P, 2, CO], bf16, kind="ExternalInput")
    out = nc.dram_tensor("out", [P, NBF, 2, BLK], bf16, kind="ExternalOutput")
    outt = nc.dram_tensor("outt", [P, 2, TAIL], bf16, kind="ExternalOutput")

    with tile.TileContext(nc) as tc, ExitStack() as ctx:
        sb = ctx.enter_context(tc.tile_pool(name="sb", bufs=1))
        pp = ctx.enter_context(tc.tile_pool(name="pp", bufs=3, space="PSUM"))

        eng = {"sync": nc.sync, "scalar": nc.scalar}

        # --- warm tile for PE-ramp dummy matmuls ---
        warm = sb.tile([P, 2 * P], bf16, tag="warm")
        nc.gpsimd.memset(warm[:], 0.0)
        wps = pp.tile([P, 2 * P], f32, tag="warm", bufs=1)

        def warmup(n):
            for _ in range(n):
                nc.tensor.matmul(wps[:], warm[:, :P], warm[:], start=True,
                                 stop=True)

        warmup(NWARM_PRE)

        # --- loads: ALL on the sync queue, criticality-ordered ---
        wt = sb.tile([P, 2, CO], bf16, tag="w")
        nc.sync.dma_start(wt[:], wd[:])
        xtt = sb.tile([P, 2, TAIL], fp8, tag="xtail")
        nc.sync.dma_start(xtt[:], xtd[:])
        xg = []
        for gi, (b0, nb) in enumerate(LGROUPS):
            xt = sb.tile([P, nb, 2, BLK], fp8, tag=f"xg{gi}", name=f"xg{gi}")
            xg.append(xt)
            nc.sync.dma_start(xt[:], xsd[:, b0:b0 + nb, :, :])

        yg = [sb.tile([P, nb, 2, BLK], bf16, tag=f"yg{gi}", name=f"yg{gi}")
              for gi, (b0, nb, e) in enumerate(SGROUPS)]
        ytt = sb.tile([P, 2, TAIL], bf16, tag="ytail")

        def drain(i, dst, src):
            if i % 2 == 0:
                nc.vector.tensor_copy(dst, src)
            else:
                nc.scalar.activation(dst, src,
                                     mybir.ActivationFunctionType.Copy)

        # --- tail block first (inputs at the queue head; small store
        # leaves the trailing path early).  One 1-bank PSUM tile holds
        # both CO chunks; one drain. ---
        pt = pp.tile([P, 2 * TAIL], f32, tag="pt", bufs=1)
        for c in range(2):
            for q in range(2):
                nc.tensor.matmul(
                    pt[:, c * TAIL:(c + 1) * TAIL],
                    wt[:, q, c * P:(c + 1) * P], xtt[:, q, :],
                    start=(q == 0), stop=(q == 1))
        nc.vector.tensor_copy(ytt[:], pt[:])
        nc.scalar.dma_start(outt[:], ytt[:])

        # --- keep the PE busy until block 0's data lands (idle gaps
        # reset the p-state ramp) ---
        warmup(NWARM_MID)

        # --- main loop over full blocks.  2-bank PSUM tile per block,
        # 4 matmuls, one drain. ---
        for b in range(NBF):
            lg = max(i for i, (b0, nb) in enumerate(LGROUPS) if b0 <= b)
            lj = b - LGROUPS[lg][0]
            sg = max(i for i, (b0, nb, e) in enumerate(SGROUPS) if b0 <= b)
            sj = b - SGROUPS[sg][0]
            ps = pp.tile([P, 2 * BLK], f32)
            for c in range(2):
                for q in range(2):
                    nc.tensor.matmul(
                        ps[:, c * BLK:(c + 1) * BLK],
                        wt[:, q, c * P:(c + 1) * P],
                        xg[lg][:, lj, q, :],
                        start=(q == 0), stop=(q == 1))
            drain(b, yg[sg][:, sj, :, :], ps[:])
            if sj == SGROUPS[sg][1] - 1:
                b0, nb, e = SGROUPS[sg]
                eng[e].dma_start(out[:, b0:b0 + nb, :, :], yg[sg][:])

    nc.finalize()
    return nc


def kernel(x, symm_norm, domains, w, b):
    x = np.asarray(x, dtype=np.float32)
    symm_norm = np.asarray(symm_norm, dtype=np.float32)
    domains = np.asarray(domains)
    w = np.asarray(w, dtype=np.float32)
    b = np.asarray(b, dtype=np.float32)
    assert np.all(b == 0.0), "kernel built for b == 0 (reference uses zeros)"

    # host marshalling: fold symm_norm + doubled channels; x -> fp8 e3m4
    import ml_dtypes
    bf = ml_dtypes.bfloat16
    f8 = ml_dtypes.float8_e3m4
    xs = (x * symm_norm[:, None]).astype(f8)               # [N, C]
    w_eff = (w[:C] + w[C:]).astype(bf)                     # [C, CO]
    # w layout [p, q, co] = w_eff[q*128+p, co]
    wdev = np.ascontiguousarray(w_eff.reshape(2, P, CO).transpose(1, 0, 2))

    in_maps = []
    for c in range(NCORES):
        shard = np.zeros((R, C), dtype=f8)
        shard[:RPC] = xs[c * RPC:(c + 1) * RPC]
        # main [p, b, q, r] = xs[base + b*512 + r, q*128 + p]
        xdev = np.ascontiguousarray(
            shard[:NBF * BLK].reshape(NBF, BLK, 2, P).transpose(3, 0, 2, 1))
        # tail [p, q, r] = xs[base + 6144 + r, q*128 + p]
        xtail = np.ascontiguousarray(
            shard[NBF * BLK:].reshape(TAIL, 2, P).transpose(2, 1, 0))
        in_maps.append({"xs": xdev, "xt": xtail, "w": wdev})

    if "nc" not in _cache:
        _cache["nc"] = _build_nc()
    nc = _cache["nc"]

    res = run_bass_kernel_spmd(
        nc, in_maps, core_ids=list(range(NCORES)),
        trace=TRACE, tmpdir=TMPDIR,
    )
    _cache["last_results"] = res

    ynode = np.empty((N, CO), dtype=np.float32)
    for c, r in enumerate(res.results):
        dev = np.asarray(r["out"])                          # [p, b, coc, r]
        yc = dev.transpose(1, 3, 2, 0).reshape(NBF * BLK, CO)
        devt = np.asarray(r["outt"])                        # [p, coc, r]
        yt = devt.transpose(2, 1, 0).reshape(TAIL, CO)
        ynode[c * RPC:(c + 1) * RPC] = np.concatenate(
            [yc, yt], axis=0)[:RPC]
    # fan out: one computed row per node -> every (d, k) slot that cites it
    return ynode[domains.reshape(-1)].reshape(D, K, CO)
